# revision 1
# baseline (speedup 1.0000x reference)
"""AttnLSTMDecoder Trainium2 kernel: builder + host preprocessing.

Sharding: encoder length axis L split evenly across 8 cores; per-step
softmax normalizer + context partials all-reduced via remote SBUF DMA
broadcast (mesh all-to-all, one hop). LSTM replicated on every core.
"""
import sys
sys.path.insert(0, '/opt/trn_rl_repo')
import numpy as np
import ml_dtypes
from contextlib import ExitStack
from concourse import bass, bacc, tile
mybir = bass.mybir

F32 = mybir.dt.float32
BF16 = mybir.dt.bfloat16
Tanh = mybir.ActivationFunctionType.Tanh
Exp = mybir.ActivationFunctionType.Exp
ADD = mybir.AluOpType.add
MULT = mybir.AluOpType.mult

STATE = 100
ATT = 100
EMB = 100
VOCAB = 128
NCORES = 8


def build_kernel(Lc, T, n_tanh_chunks=4, ctx_groups=4, repeats=1, ablate_exchange=False, skip=(), wbufs=2):
    """Build the per-core SPMD kernel. Lc = L/8 (multiple of 512)."""
    NCH = Lc // 128          # l-chunks of 128
    assert Lc % 512 == 0
    assert NCH % n_tanh_chunks == 0
    assert NCH % ctx_groups == 0 or True
    nc = bacc.Bacc("TRN2", target_bir_lowering=False, debug=False,
                   num_devices=NCORES)

    # ---------------- DRAM parameters (per-core) ----------------
    d_imB = nc.declare_dram_parameter("imB", [200, Lc], F32, isOutput=False)
    d_imTa = nc.declare_dram_parameter("imTa", [128, NCH * 208], F32, isOutput=False)
    d_w1T = nc.declare_dram_parameter("w1T", [200, ATT], F32, isOutput=False)
    d_vb = nc.declare_dram_parameter("vb", [ATT, 1], BF16, isOutput=False)
    d_Wg = nc.declare_dram_parameter("Wg", [401, 400], F32, isOutput=False)
    d_w2T = nc.declare_dram_parameter("w2T", [200, ATT], F32, isOutput=False)
    d_linTb = nc.declare_dram_parameter("linTb", [101, VOCAB], F32, isOutput=False)
    d_linsel = nc.declare_dram_parameter("linsel", [101, T], F32, isOutput=False)
    d_embaug = nc.declare_dram_parameter("embaug", [101, T], F32, isOutput=False)
    d_h0 = nc.declare_dram_parameter("h0aug", [101, 1], F32, isOutput=False)
    d_c0 = nc.declare_dram_parameter("c0", [STATE, 1], F32, isOutput=False)
    d_S = nc.declare_dram_parameter("Sout", [1, T], F32, isOutput=True)
    d_sel = nc.declare_dram_parameter("selout", [1, T], F32, isOutput=True)
    d_selEO = nc.declare_dram_parameter("selEO", [16, 2], F32, isOutput=False)
    ccin = [nc.dram_tensor(f"ccin{i}", [2, 416], F32) for i in range(2)]
    ccout = [nc.dram_tensor(f"ccout{i}", [16, 416], F32, addr_space="Shared")
             for i in range(2)]

    with tile.TileContext(nc) as tc, ExitStack() as ctxs:
        # pools
        P = ctxs.enter_context(tc.tile_pool(name="static", bufs=1))
        W = ctxs.enter_context(tc.tile_pool(name="work", bufs=wbufs))
        PS = ctxs.enter_context(tc.tile_pool(name="psum", bufs=1,
                                             space="PSUM"))

        # ---------------- static SBUF tiles ----------------
        imB1 = P.tile([100, Lc], F32, tag="imB1")
        imB2 = P.tile([100, Lc], F32, tag="imB2")
        imTa = P.tile([128, NCH * 208], F32, tag="imTa")
        w1Ta = P.tile([100, ATT], F32, tag="w1Ta")
        w1Tb = P.tile([100, ATT], F32, tag="w1Tb")
        vb = P.tile([ATT, 1], BF16, tag="vb")
        Wg_ctx1 = P.tile([128, 400], F32, tag="Wgc1")
        Wg_ctx2 = P.tile([72, 400], F32, tag="Wgc2")
        Wg_embb = P.tile([101, 400], F32, tag="Wge")
        Wg_h = P.tile([100, 400], F32, tag="Wgh")
        w2Th = P.tile([100, ATT], F32, tag="w2Th")
        w2Tc = P.tile([100, ATT], F32, tag="w2Tc")
        linTb = P.tile([101, VOCAB], F32, tag="linTb")
        linsel = P.tile([101, T], F32, tag="linsel")
        embaug = P.tile([101, T], F32, tag="embaug")
        h_aug = P.tile([101, 1], F32, tag="haug")
        c_sb = P.tile([STATE, 1], F32, tag="c")
        w1tb = P.tile([ATT, Lc], BF16, tag="w1tb")
        tanh_sb = P.tile([ATT, Lc], BF16, tag="tanhsb")
        Sbuf = P.tile([1, T], F32, tag="Sbuf")
        selbuf = P.tile([1, T], F32, tag="selbuf")
        ones1 = P.tile([1, 1], F32, tag="ones1")
        ones128 = P.tile([1, 128], F32, tag="ones128")
        selEO = P.tile([16, 2], F32, tag="selEO")

        # ---------------- init ----------------
        nc.sync.dma_start(imB1[:], d_imB[0:100, :])
        nc.sync.dma_start(imB2[:], d_imB[100:200, :])
        nc.sync.dma_start(imTa[:], d_imTa[:])
        nc.sync.dma_start(w1Ta[:], d_w1T[0:100, :])
        nc.sync.dma_start(w1Tb[:], d_w1T[100:200, :])
        nc.sync.dma_start(vb[:], d_vb[:])
        nc.sync.dma_start(Wg_ctx1[:], d_Wg[0:128, :])
        nc.sync.dma_start(Wg_ctx2[:], d_Wg[128:200, :])
        nc.sync.dma_start(Wg_embb[:], d_Wg[200:301, :])
        nc.sync.dma_start(Wg_h[:], d_Wg[301:401, :])
        nc.sync.dma_start(w2Th[:], d_w2T[0:100, :])
        nc.sync.dma_start(w2Tc[:], d_w2T[100:200, :])
        nc.sync.dma_start(linTb[:], d_linTb[:])
        nc.sync.dma_start(linsel[:], d_linsel[:])
        nc.sync.dma_start(embaug[:], d_embaug[:])
        nc.sync.dma_start(h_aug[:], d_h0[:])
        nc.sync.dma_start(c_sb[:], d_c0[:])
        nc.gpsimd.memset(ones1[:], 1.0)
        nc.gpsimd.memset(ones128[:], 1.0)
        nc.sync.dma_start(selEO[:], d_selEO[:])

        # w1t = w1 @ input_mat   -> [ATT, Lc] bf16
        for j in range(Lc // 512):
            w1p = PS.tile([ATT, 512], F32, tag="w1p")
            sl = slice(512 * j, 512 * (j + 1))
            nc.tensor.matmul(w1p[:], w1Ta[:], imB1[:, sl], start=True, stop=False)
            nc.tensor.matmul(w1p[:], w1Tb[:], imB2[:, sl], start=False, stop=True)
            nc.scalar.copy(w1tb[:, sl], w1p[:])

        CH = NCH // n_tanh_chunks  # l-chunks per tanh chunk

        # ---------------- decode steps ----------------
        for t in [tt for _ in range(repeats) for tt in range(T)]:
            # w2dt = w2 @ [h; c]  -> bias for tanh
            w2p_full = PS.tile([ATT, 512], F32, tag="w2p")
            w2p = w2p_full[:, 0:1]
            nc.tensor.matmul(w2p[:], w2Th[:], h_aug[0:100, 0:1], start=True, stop=False)
            nc.tensor.matmul(w2p[:], w2Tc[:], c_sb[:], start=False, stop=True)
            bias_sb = W.tile([ATT, 1], F32, tag="bias")
            nc.scalar.copy(bias_sb[:], w2p[:])

            scores_full = PS.tile([128, 512], F32, tag="scores")
            scores_ps = scores_full[:, 0:NCH]
            att_sb = W.tile([128, NCH], F32, tag="att")
            ctx_full = PS.tile([2, 512], F32, tag="ctx")
            ctx_ps = ctx_full[:, 0:416]

            for ch in range(n_tanh_chunks):
                lo, hi = ch * CH, (ch + 1) * CH
                tlo, thi = (lo, lo + 128) if "tanh1" in skip else (lo * 128, hi * 128)
                nc.scalar.activation(tanh_sb[:, tlo:thi],
                                     w1tb[:, tlo:thi],
                                     Tanh, bias=bias_sb[:, 0:1])
                srange = [lo] if "scores1" in skip else range(lo, hi)
                for c in srange:
                    nc.tensor.matmul(scores_ps[:, c:c + 1],
                                     tanh_sb[:, c * 128:(c + 1) * 128],
                                     vb[:], start=True, stop=True)
                nc.scalar.activation(att_sb[:, lo:hi], scores_ps[:, lo:hi], Exp)
                crange = [lo] if "ctx4" in skip else range(lo, hi, 2)
                for c in crange:
                    nc.tensor.matmul(ctx_ps[:],
                                     att_sb[:, c:c + 2],
                                     imTa[:, c * 208:(c + 2) * 208],
                                     start=(c == lo if "ctx4" in skip
                                            else c == 0),
                                     stop=(c >= (hi if "ctx4" in skip
                                                 else NCH) - 2))

            # both partial half-rows go to the exchange uncombined
            num_sb = W.tile([2, 416], F32, tag="num")
            if "combine" not in skip:
                nc.vector.tensor_copy(num_sb[:], ctx_ps[:])


            # ---- exchange: AllGather the [2,416] half-rows ----
            gather = W.tile([16, 416], F32, tag="gather")
            if ablate_exchange:
                nc.vector.tensor_copy(gather[0:2, :], num_sb[:])
            else:
                cin, cout = ccin[t % 2], ccout[t % 2]
                nc.sync.dma_start(cin[:], num_sb[:])
                nc.gpsimd.collective_compute(
                    "AllGather", mybir.AluOpType.bypass,
                    replica_groups=[list(range(NCORES))],
                    ins=[cin.ap().opt()], outs=[cout.ap().opt()])
                nc.sync.dma_start(gather[:], cout[:])

            if "post" in skip:
                continue
            # reduce over ranks AND transpose to columns: even rows carry
            # cols 0:208, odd rows cols 208:416; 0/1 masks select them and
            # both halves accumulate into the same PSUM columns
            KR = 2 if ablate_exchange else 16
            sE, sO = selEO[0:KR, 0:1], selEO[0:KR, 1:2]
            cu_full = PS.tile([128, 512], F32, tag="cu")
            g_ = gather[0:KR, :]
            nc.tensor.matmul(cu_full[:, 0:1], g_[:, 1:129], sE,
                             start=True, stop=False)
            nc.tensor.matmul(cu_full[:, 0:1], g_[:, 209:337], sO,
                             start=False, stop=True)
            nc.tensor.matmul(cu_full[0:72, 1:2], g_[:, 129:201], sE,
                             start=True, stop=False)
            nc.tensor.matmul(cu_full[0:72, 1:2], g_[:, 337:409], sO,
                             start=False, stop=True)
            # den reduced AND broadcast to all 128 partitions in one matmul
            # (stationary free-dim stride 0 replicates the den column)
            nc.tensor.matmul(cu_full[:, 2:3],
                             g_[:, 0:1].to_broadcast((KR, 128)), sE,
                             start=True, stop=False)
            nc.tensor.matmul(cu_full[:, 2:3],
                             g_[:, 208:209].to_broadcast((KR, 128)), sO,
                             start=False, stop=True)
            rd = W.tile([128, 1], F32, tag="rd")
            nc.vector.reciprocal(rd[:], cu_full[:, 2:3])
            ctx_sb = W.tile([128, 2], F32, tag="ctxs")
            nc.scalar.activation(ctx_sb[:, 0:1], cu_full[:, 0:1],
                                 mybir.ActivationFunctionType.Copy,
                                 scale=rd[:, 0:1])
            nc.scalar.activation(ctx_sb[0:72, 1:2], cu_full[0:72, 1:2],
                                 mybir.ActivationFunctionType.Copy,
                                 scale=rd[0:72, 0:1])

            # gates = Wg @ [ctx(200); emb;1; h]  -> [100, 4] (i,f,o pre-scaled 0.5)
            gates_full = PS.tile([100, 512], F32, tag="gates")
            gates_ps = gates_full[:, 0:4]
            for g in range(4 if "gates" not in skip else 0):
                gs = slice(100 * g, 100 * (g + 1))
                nc.tensor.matmul(gates_ps[:, g:g + 1], Wg_embb[:, gs],
                                 embaug[:, t:t + 1], start=True, stop=False)
                nc.tensor.matmul(gates_ps[:, g:g + 1], Wg_h[:, gs],
                                 h_aug[0:100, 0:1], start=False, stop=False)
                nc.tensor.matmul(gates_ps[:, g:g + 1], Wg_ctx1[:, gs],
                                 ctx_sb[:, 0:1], start=False, stop=False)
                nc.tensor.matmul(gates_ps[:, g:g + 1], Wg_ctx2[:, gs],
                                 ctx_sb[0:72, 1:2], start=False, stop=True)

            # LSTM elementwise
            t_all = W.tile([100, 4], F32, tag="tall")
            if "lstm" in skip:
                continue
            nc.scalar.activation(t_all[:], gates_ps[:], Tanh)
            sig = W.tile([100, 3], F32, tag="sig")
            nc.vector.tensor_scalar(sig[:], t_all[:, 0:3], 1.0, 0.5, ADD, MULT)
            tmp1 = W.tile([100, 1], F32, tag="tmp1")
            tmp2 = W.tile([100, 1], F32, tag="tmp2")
            nc.vector.tensor_tensor(tmp1[:], sig[:, 1:2], c_sb[:], op=MULT)
            nc.vector.tensor_tensor(tmp2[:], sig[:, 0:1], t_all[:, 3:4], op=MULT)
            nc.vector.tensor_tensor(c_sb[:], tmp1[:], tmp2[:], op=ADD)
            tanh_c = W.tile([100, 1], F32, tag="tanhc")
            nc.scalar.activation(tanh_c[:], c_sb[:], Tanh)
            nc.vector.tensor_tensor(h_aug[0:100, 0:1], sig[:, 2:3], tanh_c[:],
                                    op=MULT)

            # logits + per-step loss pieces
            if "logits" in skip:
                continue
            lg_full = PS.tile([1, 512], F32, tag="lg")
            lg_ps = lg_full[:, 0:129]
            nc.tensor.matmul(lg_ps[0:1, 0:128], h_aug[:, 0:1], linTb[:],
                             start=True, stop=True)
            nc.tensor.matmul(lg_ps[0:1, 128:129], h_aug[:, 0:1],
                             linsel[:, t:t + 1], start=True, stop=True)
            exps = W.tile([1, VOCAB], F32, tag="exps")
            nc.scalar.activation(exps[:], lg_ps[0:1, 0:128], Exp,
                                 accum_out=Sbuf[0:1, t:t + 1])
            nc.vector.tensor_copy(selbuf[0:1, t:t + 1], lg_ps[0:1, 128:129])

        nc.sync.dma_start(d_S[:], Sbuf[:])
        nc.sync.dma_start(d_sel[:], selbuf[:])

    return nc


# =================== host preprocessing ===================

def _lstm_step_np(x, h, c, W_ih, W_hh, b_ih, b_hh):
    gates = W_ih @ x + b_ih + W_hh @ h + b_hh
    i, f, g, o = np.split(gates, 4)
    sig = lambda v: 1.0 / (1.0 + np.exp(-v))
    c = sig(f) * c + sig(i) * np.tanh(g)
    h = sig(o) * np.tanh(c)
    return h, c


def prep_inputs(inputs, Lc, T):
    """Produce the 8 per-core in_maps from the full problem inputs."""
    im = np.asarray(inputs["input_mat"], np.float32)        # [200, L]
    output_ids = np.asarray(inputs["output_ids"]).astype(np.int64)
    W_ih = np.asarray(inputs["W_ih"], np.float32)
    W_hh = np.asarray(inputs["W_hh"], np.float32)
    b_ih = np.asarray(inputs["b_ih"], np.float32)
    b_hh = np.asarray(inputs["b_hh"], np.float32)
    w1 = np.asarray(inputs["w1"], np.float32)
    w2 = np.asarray(inputs["w2"], np.float32)
    v_w = np.asarray(inputs["v_w"], np.float32)
    lin_w = np.asarray(inputs["lin_w"], np.float32)
    lin_b = np.asarray(inputs["lin_b"], np.float32)
    emb = np.asarray(inputs["emb"], np.float32)
    eos = int(np.asarray(inputs["eos_id"]))

    L = im.shape[1]
    assert Lc * NCORES == L and len(output_ids) == T
    NCH = Lc // 128

    # priming LSTM step on host (exact fp32 math, tiny)
    x0 = np.concatenate([np.zeros(200, np.float32), emb[eos]])
    h0, c0 = _lstm_step_np(x0, np.zeros(100, np.float32),
                           np.zeros(100, np.float32), W_ih, W_hh, b_ih, b_hh)
    h0aug = np.concatenate([h0, [1.0]]).astype(np.float32).reshape(101, 1)

    # gates weights: reorder [i,f,g,o] -> [i,f,o,g], scale i,f,o by 0.5,
    # columns [ctx(200); emb(100); bias(1); h(100)]; transposed for lhsT.
    Wcomb = np.concatenate([W_ih, W_hh], axis=1)            # [400, 400]
    bias = (b_ih + b_hh).astype(np.float32)                 # [400]
    order = np.concatenate([np.arange(100), np.arange(100, 200),
                            np.arange(300, 400), np.arange(200, 300)])
    Wr = Wcomb[order]                                       # rows i,f,o,g
    br = bias[order].copy()
    scale = np.ones((400, 1), np.float32); scale[0:300] = 0.5
    Wr = Wr * scale; br = br * scale[:, 0]
    Wg = np.zeros((401, 400), np.float32)
    # Wcomb columns = [x(300) ; h(100)] where x = [ctx(200); emb(100)]
    Wg[0:200] = Wr[:, 0:200].T       # ctx
    Wg[200:300] = Wr[:, 200:300].T   # emb
    Wg[300] = br                     # bias row
    Wg[301:401] = Wr[:, 300:400].T   # h
    # emb sequence (last_emb per step) with bias-1 row
    emb_seq = np.empty((T, EMB), np.float32)
    emb_seq[0] = emb[eos]
    emb_seq[1:] = emb[output_ids[:T - 1]]
    embaug = np.concatenate([emb_seq.T, np.ones((1, T), np.float32)], axis=0)
    # logits weights
    linTb = np.concatenate([lin_w.T, lin_b.reshape(1, -1)], axis=0)  # [101,128]
    linsel = np.concatenate([lin_w[output_ids[:T]].T,
                             lin_b[output_ids[:T]].reshape(1, -1)], axis=0)

    w1T = w1.T.copy()                 # [200, 100]
    w2T = w2.T.copy()                 # [200, 100]
    vbf = v_w.reshape(ATT, 1).astype(ml_dtypes.bfloat16)

    in_maps = []
    for cidx in range(NCORES):
        sl = slice(cidx * Lc, (cidx + 1) * Lc)
        imc = im[:, sl]                                    # [200, Lc]
        # imTa[q, c*201 + s] = im[s, c*128+q]; col 200 = 1.0
        blocks = imc.T.reshape(NCH, 128, 200)              # [c, q, s]
        imTa = np.concatenate(
            [np.ones((NCH, 128, 1), np.float32), blocks,
             np.zeros((NCH, 128, 7), np.float32)], axis=2)  # [c,q,208]
        imTa = imTa.transpose(1, 0, 2).reshape(128, NCH * 208).copy()
        selEO = np.zeros((16, 2), np.float32)
        selEO[0::2, 0] = 1.0
        selEO[1::2, 1] = 1.0
        in_maps.append({
            "selEO": selEO,
            "imB": np.ascontiguousarray(imc),
            "imTa": imTa,
            "w1T": w1T, "vb": vbf, "Wg": Wg, "w2T": w2T,
            "linTb": linTb.astype(np.float32),
            "linsel": linsel.astype(np.float32),
            "embaug": embaug.astype(np.float32),
            "h0aug": h0aug, "c0": c0.reshape(100, 1).astype(np.float32),
        })
    return in_maps


def finish_loss(Sout, selout):
    """loss = sum_t ( log(sum_j exp(logit_j)) - logit_sel )."""
    S = np.asarray(Sout, np.float64).ravel()
    sel = np.asarray(selout, np.float64).ravel()
    return np.float32(np.sum(np.log(S) - sel))


# =================== self-contained runner ===================
LC = 8192
T_STEPS = 258
_CACHE = {}


def _get_compiled():
    if "nc" not in _CACHE:
        nc = build_kernel(LC, T_STEPS)
        nc.compile()
        _CACHE["nc"] = nc
    return _CACHE["nc"]


def kernel(**inputs):
    """Full-input AttnLSTM decoder loss on 8 trn2 cores."""
    from concourse import bass_utils
    nc = _get_compiled()
    in_maps = prep_inputs(inputs, LC, T_STEPS)
    res = bass_utils.run_bass_kernel_spmd(nc, in_maps,
                                          core_ids=list(range(NCORES)))
    out = res.results[0]
    return np.asarray(finish_loss(out["Sout"], out["selout"]))



# revision 26
# speedup vs baseline: 1102.4274x; 1102.4274x over previous
"""AttnLSTMDecoder Trainium2 kernel: builder + host preprocessing.

Sharding: encoder length axis L split evenly across 8 cores. Per-step the
softmax numerator/denominator partials are exchanged with direct SBUF->SBUF
remote DMA broadcasts (XOR-relative peers, one slot per sender) instead of a
firmware collective; each core then reduces the 8 slots locally. The LSTM is
replicated on every core.

Key optimizations over the collective baseline:
- ctx matvec streams bf16 (4x PE throughput vs fp32)
- remote_dma exchange (~2us) replaces AllGather via DRAM (~17us modeled)
- gate preactivations from the embedding path precomputed on host
- LSTM elementwise fused via scalar_tensor_tensor with 2x-scaled c/h
"""
import sys
sys.path.insert(0, '/opt/trn_rl_repo')
import numpy as np
import ml_dtypes
from contextlib import ExitStack
from concourse import bass, bacc, tile
mybir = bass.mybir

F32 = mybir.dt.float32
BF16 = mybir.dt.bfloat16
Tanh = mybir.ActivationFunctionType.Tanh
Exp = mybir.ActivationFunctionType.Exp
Copy = mybir.ActivationFunctionType.Copy
ADD = mybir.AluOpType.add
MULT = mybir.AluOpType.mult

STATE = 100
ATT = 100
EMB = 100
VOCAB = 128
NCORES = 8


def build_kernel(Lc, T, n_tanh_chunks=4, repeats=1, exchange="rdma", skip=(),
                 wbufs=2):
    """Build the per-core SPMD kernel. Lc = L/8 (multiple of 512)."""
    NCH = Lc // 128          # l-chunks of 128
    assert Lc % 512 == 0
    assert NCH % n_tanh_chunks == 0
    nc = bacc.Bacc("TRN2", target_bir_lowering=False, debug=False,
                   num_devices=NCORES)

    # ---------------- DRAM parameters (per-core) ----------------
    d_imB = nc.declare_dram_parameter("imB", [200, Lc], F32, isOutput=False)
    d_imTa = nc.declare_dram_parameter("imTa", [128, NCH * 208], BF16,
                                       isOutput=False)
    d_w1T = nc.declare_dram_parameter("w1T", [200, ATT], F32, isOutput=False)
    d_vb = nc.declare_dram_parameter("vb", [ATT, 1], BF16, isOutput=False)
    d_Wg = nc.declare_dram_parameter("Wg", [301, 400], F32, isOutput=False)
    d_w2T = nc.declare_dram_parameter("w2T", [200, ATT], F32, isOutput=False)
    d_linTb = nc.declare_dram_parameter("linTb", [101, VOCAB], F32,
                                        isOutput=False)
    d_linsel = nc.declare_dram_parameter("linsel", [101, T], F32,
                                         isOutput=False)
    d_gEmb = nc.declare_dram_parameter("gEmb", [100, 4 * T], F32,
                                       isOutput=False)
    d_h0 = nc.declare_dram_parameter("h0aug", [101, 1], F32, isOutput=False)
    d_c0 = nc.declare_dram_parameter("c0", [STATE, 1], F32, isOutput=False)
    d_selEO = nc.declare_dram_parameter("selEO", [2, 2], BF16, isOutput=False)
    d_S = nc.declare_dram_parameter("Sout", [1, T], F32, isOutput=True)
    d_sel = nc.declare_dram_parameter("selout", [1, T], F32, isOutput=True)
    if exchange == "cc":
        ccin = [nc.dram_tensor(f"ccin{i}", [128, 3], F32) for i in range(2)]
        ccout = [nc.dram_tensor(f"ccout{i}", [1024, 3], F32,
                                addr_space="Shared") for i in range(2)]

    # parity-split remote sems: step t uses rsems[t%2] so a wait can only be
    # satisfied by same-parity arrivals (peers can be at most 1 step ahead,
    # which is the other parity — no overshoot race).
    rsems = [nc.alloc_semaphore("rdma_rsem0"), nc.alloc_semaphore("rdma_rsem1")]
    lsems = [nc.alloc_semaphore("rdma_lsem0"), nc.alloc_semaphore("rdma_lsem1")]

    with tile.TileContext(nc) as tc, ExitStack() as ctxs:
        P = ctxs.enter_context(tc.tile_pool(name="static", bufs=1))
        W = ctxs.enter_context(tc.tile_pool(name="work", bufs=wbufs))
        PS = ctxs.enter_context(tc.tile_pool(name="psum", bufs=1,
                                             space="PSUM"))

        # ---------------- static SBUF tiles ----------------
        imB1 = P.tile([100, Lc], F32, tag="imB1")
        imB2 = P.tile([100, Lc], F32, tag="imB2")
        imTa = P.tile([128, NCH * 208], BF16, tag="imTa")
        w1Ta = P.tile([100, ATT], F32, tag="w1Ta")
        w1Tb = P.tile([100, ATT], F32, tag="w1Tb")
        vb = P.tile([ATT, 1], BF16, tag="vb")
        Wg_ctxA = P.tile([128, 400], F32, tag="WgcA")
        Wg_ctxB = P.tile([72, 400], F32, tag="WgcB")
        Wg_hb = P.tile([101, 400], F32, tag="Wghb")
        w2Th = P.tile([100, ATT], F32, tag="w2Th")
        w2Tc = P.tile([100, ATT], F32, tag="w2Tc")
        linTb = P.tile([101, VOCAB], F32, tag="linTb")
        linsel = P.tile([101, T], F32, tag="linsel")
        gEmb = P.tile([100, 4 * T], F32, tag="gEmb")
        h_aug = P.tile([101, 1], F32, tag="haug")
        c_sb = P.tile([STATE, 1], F32, tag="c")
        w1tb = P.tile([ATT, Lc], BF16, tag="w1tb")
        tanh_sb = P.tile([ATT, Lc], BF16, tag="tanhsb")
        Sbuf = P.tile([1, T], F32, tag="Sbuf")
        selbuf = P.tile([1, T], F32, tag="selbuf")
        selEO = P.tile([2, 2], BF16, tag="selEO")
        comm = [P.tile([128, 3], F32, tag=f"comm{i}", name=f"comm{i}")
                for i in range(2)]
        gather = [P.tile([128, 24], F32, tag=f"gather{i}", name=f"gather{i}")
                  for i in range(2)]

        # ---------------- init ----------------
        nc.sync.dma_start(imB1[:], d_imB[0:100, :])
        nc.sync.dma_start(imB2[:], d_imB[100:200, :])
        nc.sync.dma_start(imTa[:], d_imTa[:])
        nc.sync.dma_start(w1Ta[:], d_w1T[0:100, :])
        nc.sync.dma_start(w1Tb[:], d_w1T[100:200, :])
        nc.sync.dma_start(vb[:], d_vb[:])
        nc.sync.dma_start(Wg_ctxA[:], d_Wg[0:128, :])
        nc.sync.dma_start(Wg_ctxB[:], d_Wg[128:200, :])
        nc.sync.dma_start(Wg_hb[:], d_Wg[200:301, :])
        nc.sync.dma_start(w2Th[:], d_w2T[0:100, :])
        nc.sync.dma_start(w2Tc[:], d_w2T[100:200, :])
        nc.sync.dma_start(linTb[:], d_linTb[:])
        nc.sync.dma_start(linsel[:], d_linsel[:])
        nc.sync.dma_start(gEmb[:], d_gEmb[:])
        nc.sync.dma_start(h_aug[:], d_h0[:])
        nc.sync.dma_start(c_sb[:], d_c0[:])
        nc.sync.dma_start(selEO[:], d_selEO[:])
        nc.gpsimd.memset(comm[0][:], 0.0)
        nc.gpsimd.memset(comm[1][:], 0.0)

        # w1t = w1 @ input_mat   -> [ATT, Lc] bf16
        for j in range(Lc // 512):
            w1p = PS.tile([ATT, 512], F32, tag="w1p")
            sl = slice(512 * j, 512 * (j + 1))
            nc.tensor.matmul(w1p[:], w1Ta[:], imB1[:, sl], start=True,
                             stop=False)
            nc.tensor.matmul(w1p[:], w1Tb[:], imB2[:, sl], start=False,
                             stop=True)
            nc.scalar.copy(w1tb[:, sl], w1p[:])

        CH = NCH // n_tanh_chunks  # l-chunks per tanh chunk
        sE, sO = selEO[0:2, 0:1], selEO[0:2, 1:2]

        # ---------------- decode steps ----------------
        step = 0
        for t in [tt for _ in range(repeats) for tt in range(T)]:
            p = step % 2
            # w2dt = w2 @ [h; c]  -> bias for tanh (read directly from PSUM)
            w2p_full = PS.tile([ATT, 512], F32, tag="w2p")
            w2p = w2p_full[:, 0:1]
            nc.tensor.matmul(w2p[:], w2Th[:], h_aug[0:100, 0:1], start=True,
                             stop=False)
            nc.tensor.matmul(w2p[:], w2Tc[:], c_sb[:], start=False, stop=True)
            bias_sb = W.tile([ATT, 1], F32, tag="bias")
            nc.scalar.copy(bias_sb[:], w2p[:])

            scores_full = PS.tile([128, 512], F32, tag="scores")
            scores_ps = scores_full[:, 0:NCH]
            att_sb = W.tile([128, NCH], BF16, tag="att")
            ctx_full = PS.tile([2, 512], F32, tag="ctx")
            ctx_ps = ctx_full[:, 0:416]

            for ch in range(n_tanh_chunks):
                lo, hi = ch * CH, (ch + 1) * CH
                tlo, thi = (lo, lo + 128) if "tanh1" in skip else (lo * 128,
                                                                   hi * 128)
                nc.scalar.activation(tanh_sb[:, tlo:thi], w1tb[:, tlo:thi],
                                     Tanh, bias=bias_sb[:, 0:1])
                srange = [lo] if "scores1" in skip else range(lo, hi)
                for c in srange:
                    nc.tensor.matmul(scores_ps[:, c:c + 1],
                                     tanh_sb[:, c * 128:(c + 1) * 128],
                                     vb[:], start=True, stop=True)
                nc.scalar.activation(att_sb[:, lo:hi], scores_ps[:, lo:hi],
                                     Exp)
                crange = [lo] if "ctx4" in skip else range(lo, hi, 2)
                for c in crange:
                    nc.tensor.matmul(ctx_ps[:],
                                     att_sb[:, c:c + 2],
                                     imTa[:, c * 208:(c + 2) * 208],
                                     start=(c == lo if "ctx4" in skip
                                            else c == 0),
                                     stop=(c >= (hi if "ctx4" in skip
                                                 else NCH) - 2))

            if "post" in skip:
                step += 1
                continue

            # partial [2, 416] -> column layout [128, 4] (reduce E/O + transpose)
            # cu columns: 0 = num[0:128], 1 = den (bcast), 2 = num[128:200]
            # (den in the middle so the initialized region is col 0:2 full
            # plus col 2 rows 0:72 — contiguous slices for the comm copy)
            num_sb = W.tile([2, 416], BF16, tag="num")
            nc.vector.tensor_copy(num_sb[:], ctx_ps[:])
            cu_full = PS.tile([128, 512], F32, tag="cu")
            cu = cu_full[:, 0:3]
            nc.tensor.matmul(cu[:, 0:1], num_sb[:, 1:129], sE,
                             start=True, stop=False)
            nc.tensor.matmul(cu[:, 0:1], num_sb[:, 209:337], sO,
                             start=False, stop=True)
            nc.tensor.matmul(cu[:, 1:2], num_sb[:, 0:1].to_broadcast((2, 128)),
                             sE, start=True, stop=False)
            nc.tensor.matmul(cu[:, 1:2],
                             num_sb[:, 208:209].to_broadcast((2, 128)),
                             sO, start=False, stop=True)
            nc.tensor.matmul(cu[0:72, 2:3], num_sb[:, 129:201], sE,
                             start=True, stop=False)
            nc.tensor.matmul(cu[0:72, 2:3], num_sb[:, 337:409], sO,
                             start=False, stop=True)

            # ---- exchange: broadcast own [128, 4] partial to all 8 cores ----
            if exchange == "rdma":
                with tc.tile_critical(name="commw"):
                    # comm[p] reusable once step-(t-2) sends (same parity,
                    # 8 broadcasts x16 local_sem) completed
                    cp = nc.vector.tensor_copy(comm[p][:, 0:2], cu[:, 0:2])
                    if step >= 2:
                        # same-parity sends of step t-2 all completed locally
                        cp._wait_ge(lsems[p], 128 * (step // 2))
                    nc.vector.tensor_copy(comm[p][0:72, 2:3], cu[0:72, 2:3])
                # descgen runs early on Pool (no-sync deps); the comm-tile
                # read is deferred to trigger_dma, which waits for the copy
                for d in range(8):
                    nc.gpsimd.remote_dma_broadcast(
                        gather[p][:, 3 * d:3 * d + 3], comm[p][:],
                        rsems[p], lsems[p],
                        rdests=[(0, d) if k == d else None for k in range(8)])
                nc.gpsimd.trigger_dma(count=None)
                g = gather[p]
                r12 = W.tile([128, 12], F32, tag="r12")
                with tc.tile_critical(name="redw"):
                    # gate the first reduce on all 8 peers' arrivals
                    a1 = nc.vector.tensor_tensor(r12[:], g[:, 0:12],
                                                 g[:, 12:24], op=ADD)
                    a1._wait_ge(rsems[p], 16 * (step // 2 + 1))
            elif exchange == "cc":
                commt = W.tile([128, 3], F32, tag="commt")
                nc.scalar.activation(commt[:, 0:2], cu[:, 0:2], Copy)
                nc.scalar.activation(commt[0:72, 2:3], cu[0:72, 2:3], Copy)
                nc.gpsimd.memset(commt[72:128, 2:3], 0.0)
                cin, cout = ccin[p], ccout[p]
                nc.sync.dma_start(cin[:], commt[:])
                nc.gpsimd.collective_compute(
                    "AllGather", mybir.AluOpType.bypass,
                    replica_groups=[list(range(NCORES))],
                    ins=[cin.ap().opt()], outs=[cout.ap().opt()])
                g = W.tile([128, 24], F32, tag="gatherw")
                for d in range(8):
                    nc.sync.dma_start(g[:, 3 * d:3 * d + 3],
                                      cout[128 * d:128 * (d + 1), :])
            else:  # exchange == "none" (ablation: pretend sum == 8x local)
                g = W.tile([128, 24], F32, tag="gatherw")
                nc.gpsimd.memset(g[:], 0.0)
                for d in range(8):
                    nc.scalar.activation(g[:, 3 * d:3 * d + 2], cu[:, 0:2],
                                         Copy)

            # reduce the 8 slots -> [128, 3]; cols: ctxA, ctxB, den
            if exchange != "rdma":
                r12 = W.tile([128, 12], F32, tag="r12")
                nc.vector.tensor_tensor(r12[:], g[:, 0:12], g[:, 12:24],
                                        op=ADD)
            r6 = W.tile([128, 6], F32, tag="r6")
            nc.vector.tensor_tensor(r6[:], r12[:, 0:6], r12[:, 6:12], op=ADD)
            cu_tot = W.tile([128, 3], F32, tag="cutot")
            nc.vector.tensor_tensor(cu_tot[:], r6[:, 0:3], r6[:, 3:6], op=ADD)
            rd = W.tile([128, 1], F32, tag="rd")
            nc.vector.reciprocal(rd[:], cu_tot[:, 1:2])
            ctx_sb = W.tile([128, 3], F32, tag="ctxs")
            nc.scalar.activation(ctx_sb[:], cu_tot[:], Copy,
                                 scale=rd[:, 0:1])

            # gates = Wg @ [ctx(200); h; 1]  -> [100, 4] (i,f,o pre-scaled .5)
            gates_full = PS.tile([100, 512], F32, tag="gates")
            gates_ps = gates_full[:, 0:4]
            for gi in range(4 if "gates" not in skip else 0):
                gs = slice(100 * gi, 100 * (gi + 1))
                nc.tensor.matmul(gates_ps[:, gi:gi + 1], Wg_ctxA[:, gs],
                                 ctx_sb[:, 0:1], start=True, stop=False)
                nc.tensor.matmul(gates_ps[:, gi:gi + 1], Wg_ctxB[:, gs],
                                 ctx_sb[0:72, 2:3], start=False, stop=False)
                nc.tensor.matmul(gates_ps[:, gi:gi + 1], Wg_hb[:, gs],
                                 h_aug[:], start=False, stop=True)

            if "lstm" in skip:
                step += 1
                continue
            # LSTM elementwise; c_sb/h hold 2x-scaled state
            gsum = W.tile([100, 4], F32, tag="gsum")
            nc.vector.tensor_tensor(gsum[:], gates_ps[:],
                                    gEmb[:, 4 * t:4 * t + 4], op=ADD)
            t_all = W.tile([100, 4], F32, tag="tall")
            nc.scalar.activation(t_all[:], gsum[:], Tanh)
            tA = W.tile([100, 1], F32, tag="tA")
            nc.vector.scalar_tensor_tensor(tA[:], t_all[:, 1:2], 1.0, c_sb[:],
                                           ADD, MULT)
            tB = W.tile([100, 1], F32, tag="tB")
            nc.vector.scalar_tensor_tensor(tB[:], t_all[:, 0:1], 1.0,
                                           t_all[:, 3:4], ADD, MULT)
            nc.vector.scalar_tensor_tensor(c_sb[:], tA[:], 0.5, tB[:],
                                           MULT, ADD)
            tanh_c = W.tile([100, 1], F32, tag="tanhc")
            nc.scalar.activation(tanh_c[:], c_sb[:], Tanh, scale=0.5)
            nc.vector.scalar_tensor_tensor(h_aug[0:100, 0:1], t_all[:, 2:3],
                                           1.0, tanh_c[:], ADD, MULT)

            # logits + per-step loss pieces (off the h/c critical path)
            if "logits" not in skip:
                lg_full = PS.tile([1, 512], F32, tag="lg")
                lg_ps = lg_full[:, 0:129]
                nc.tensor.matmul(lg_ps[0:1, 0:128], h_aug[:, 0:1], linTb[:],
                                 start=True, stop=True)
                nc.tensor.matmul(lg_ps[0:1, 128:129], h_aug[:, 0:1],
                                 linsel[:, t:t + 1], start=True, stop=True)
                exps = W.tile([1, VOCAB], F32, tag="exps")
                nc.scalar.activation(exps[:], lg_ps[0:1, 0:128], Exp,
                                     accum_out=Sbuf[0:1, t:t + 1])
                nc.vector.tensor_copy(selbuf[0:1, t:t + 1],
                                      lg_ps[0:1, 128:129])
            step += 1

        nc.sync.dma_start(d_S[:], Sbuf[:])
        nc.sync.dma_start(d_sel[:], selbuf[:])

    return nc


# =================== host preprocessing ===================

def _lstm_step_np(x, h, c, W_ih, W_hh, b_ih, b_hh):
    gates = W_ih @ x + b_ih + W_hh @ h + b_hh
    i, f, g, o = np.split(gates, 4)
    sig = lambda v: 1.0 / (1.0 + np.exp(-v))
    c = sig(f) * c + sig(i) * np.tanh(g)
    h = sig(o) * np.tanh(c)
    return h, c


def prep_inputs(inputs, Lc, T):
    """Produce the 8 per-core in_maps from the full problem inputs."""
    im = np.asarray(inputs["input_mat"], np.float32)        # [200, L]
    output_ids = np.asarray(inputs["output_ids"]).astype(np.int64)
    W_ih = np.asarray(inputs["W_ih"], np.float32)
    W_hh = np.asarray(inputs["W_hh"], np.float32)
    b_ih = np.asarray(inputs["b_ih"], np.float32)
    b_hh = np.asarray(inputs["b_hh"], np.float32)
    w1 = np.asarray(inputs["w1"], np.float32)
    w2 = np.asarray(inputs["w2"], np.float32)
    v_w = np.asarray(inputs["v_w"], np.float32)
    lin_w = np.asarray(inputs["lin_w"], np.float32)
    lin_b = np.asarray(inputs["lin_b"], np.float32)
    emb = np.asarray(inputs["emb"], np.float32)
    eos = int(np.asarray(inputs["eos_id"]))

    L = im.shape[1]
    assert Lc * NCORES == L and len(output_ids) == T
    NCH = Lc // 128

    # priming LSTM step on host (exact fp32 math, tiny); store 2x-scaled
    x0 = np.concatenate([np.zeros(200, np.float32), emb[eos]])
    h0, c0 = _lstm_step_np(x0, np.zeros(100, np.float32),
                           np.zeros(100, np.float32), W_ih, W_hh, b_ih, b_hh)
    h0aug = np.concatenate([2.0 * h0, [1.0]]).astype(np.float32)
    h0aug = h0aug.reshape(101, 1)
    c0s = (2.0 * c0).reshape(100, 1).astype(np.float32)

    # gate weights: reorder [i,f,g,o] -> [i,f,o,g]; scale i,f,o rows by 0.5
    # (tanh-form sigmoid); h columns additionally by 0.5 (h,c stored 2x).
    Wcomb = np.concatenate([W_ih, W_hh], axis=1)            # [400, 400]
    bias = (b_ih + b_hh).astype(np.float32)                 # [400]
    order = np.concatenate([np.arange(100), np.arange(100, 200),
                            np.arange(300, 400), np.arange(200, 300)])
    Wr = Wcomb[order]                                       # rows i,f,o,g
    br = bias[order].copy()
    rscale = np.ones((400, 1), np.float32); rscale[0:300] = 0.5
    Wr = Wr * rscale; br = br * rscale[:, 0]
    Wr[:, 300:400] *= 0.5                                   # h stored as 2h
    # emb-path gate preactivations per step: [100, 4] blocks (i,f,o,g cols)
    emb_seq = np.empty((T, EMB), np.float32)
    emb_seq[0] = emb[eos]
    emb_seq[1:] = emb[output_ids[:T - 1]]
    gEmbAll = emb_seq @ Wr[:, 200:300].T + br               # [T, 400]
    gEmb = gEmbAll.reshape(T, 4, 100).transpose(2, 0, 1)    # [100, T, 4]
    gEmb = np.ascontiguousarray(gEmb.reshape(100, 4 * T), np.float32)
    # device-side gates input x = [ctx(200); h2(100); 1]
    Wg = np.zeros((301, 400), np.float32)
    Wg[0:200] = Wr[:, 0:200].T       # ctx
    Wg[200:300] = Wr[:, 300:400].T   # h (2x-scaled)
    Wg[300] = 0.0                    # bias handled in gEmb
    # logits weights; h rows scaled 0.5 (h stored 2x)
    linTb = np.concatenate([0.5 * lin_w.T, lin_b.reshape(1, -1)], axis=0)
    linsel = np.concatenate([0.5 * lin_w[output_ids[:T]].T,
                             lin_b[output_ids[:T]].reshape(1, -1)], axis=0)

    w1T = w1.T.copy()                                  # [200, 100]
    w2T = (0.5 * w2.T).copy()                          # [200, 100]; h,c are 2x
    vbf = v_w.reshape(ATT, 1).astype(ml_dtypes.bfloat16)
    selEO = np.zeros((2, 2), np.float32)
    selEO[0, 0] = 1.0
    selEO[1, 1] = 1.0

    in_maps = []
    for cidx in range(NCORES):
        sl = slice(cidx * Lc, (cidx + 1) * Lc)
        imc = im[:, sl]                                    # [200, Lc]
        # imTa[q, c*208 + s] = im[s, c*128+q]; col 0 = 1.0 (den)
        blocks = imc.T.reshape(NCH, 128, 200)              # [c, q, s]
        imTa = np.concatenate(
            [np.ones((NCH, 128, 1), np.float32), blocks,
             np.zeros((NCH, 128, 7), np.float32)], axis=2)  # [c,q,208]
        imTa = imTa.transpose(1, 0, 2).reshape(128, NCH * 208)
        in_maps.append({
            "selEO": selEO.astype(ml_dtypes.bfloat16),
            "imB": np.ascontiguousarray(imc),
            "imTa": np.ascontiguousarray(imTa).astype(ml_dtypes.bfloat16),
            "w1T": w1T, "vb": vbf, "Wg": Wg, "w2T": w2T,
            "linTb": linTb.astype(np.float32),
            "linsel": linsel.astype(np.float32),
            "gEmb": gEmb,
            "h0aug": h0aug, "c0": c0s,
        })
    return in_maps


def finish_loss(Sout, selout):
    """loss = sum_t ( log(sum_j exp(logit_j)) - logit_sel )."""
    S = np.asarray(Sout, np.float64).ravel()
    sel = np.asarray(selout, np.float64).ravel()
    return np.float32(np.sum(np.log(S) - sel))


# =================== self-contained runner ===================
LC = 8192
T_STEPS = 258
_CACHE = {}


def _get_compiled():
    if "nc" not in _CACHE:
        nc = build_kernel(LC, T_STEPS)
        nc.compile()
        _CACHE["nc"] = nc
    return _CACHE["nc"]


def kernel(**inputs):
    """Full-input AttnLSTM decoder loss on 8 trn2 cores."""
    from concourse import bass_utils
    nc = _get_compiled()
    in_maps = prep_inputs(inputs, LC, T_STEPS)
    res = bass_utils.run_bass_kernel_spmd(nc, in_maps,
                                          core_ids=list(range(NCORES)))
    out = res.results[0]
    return np.asarray(finish_loss(out["Sout"], out["selout"]))


# revision 27
# speedup vs baseline: 1967.6399x; 1.7848x over previous
"""AttnLSTMDecoder Trainium2 kernel: builder + host preprocessing.

Sharding: encoder length axis L split evenly across 8 cores. Per-step the
softmax numerator/denominator partials are exchanged with direct SBUF->SBUF
remote DMA broadcasts (XOR-relative peers, one slot per sender) instead of a
firmware collective; each core then reduces the 8 slots locally. The LSTM is
replicated on every core.

Key optimizations over the collective baseline:
- ctx matvec streams bf16 (4x PE throughput vs fp32)
- remote_dma exchange (~2us) replaces AllGather via DRAM (~17us modeled)
- gate preactivations from the embedding path precomputed on host
- LSTM elementwise fused via scalar_tensor_tensor with 2x-scaled c/h
"""
import sys
sys.path.insert(0, '/opt/trn_rl_repo')
import numpy as np
import ml_dtypes
from contextlib import ExitStack
from concourse import bass, bacc, tile
mybir = bass.mybir

F32 = mybir.dt.float32
BF16 = mybir.dt.bfloat16
Tanh = mybir.ActivationFunctionType.Tanh
Exp = mybir.ActivationFunctionType.Exp
Copy = mybir.ActivationFunctionType.Copy
ADD = mybir.AluOpType.add
MULT = mybir.AluOpType.mult

STATE = 100
ATT = 100
EMB = 100
VOCAB = 128
NCORES = 8


def build_kernel(Lc, T, n_tanh_chunks=4, repeats=1, exchange="rdma", skip=(),
                 wbufs=2):
    """Build the per-core SPMD kernel. Lc = L/8 (multiple of 512)."""
    NCH = Lc // 128          # l-chunks of 128
    assert Lc % 512 == 0
    assert NCH % n_tanh_chunks == 0
    nc = bacc.Bacc("TRN2", target_bir_lowering=False, debug=False,
                   num_devices=NCORES)

    # ---------------- DRAM parameters (per-core) ----------------
    d_imB = nc.declare_dram_parameter("imB", [200, Lc], F32, isOutput=False)
    d_imTa = nc.declare_dram_parameter("imTa", [128, NCH * 208], BF16,
                                       isOutput=False)
    d_w1T = nc.declare_dram_parameter("w1T", [200, ATT], F32, isOutput=False)
    d_vb = nc.declare_dram_parameter("vb", [ATT, 1], BF16, isOutput=False)
    d_Wg = nc.declare_dram_parameter("Wg", [301, 400], F32, isOutput=False)
    d_w2T = nc.declare_dram_parameter("w2T", [200, ATT], F32, isOutput=False)
    d_linTb = nc.declare_dram_parameter("linTb", [101, VOCAB], F32,
                                        isOutput=False)
    d_linsel = nc.declare_dram_parameter("linsel", [101, T], F32,
                                         isOutput=False)
    d_gEmb = nc.declare_dram_parameter("gEmb", [100, 4 * T], F32,
                                       isOutput=False)
    d_h0 = nc.declare_dram_parameter("h0aug", [101, 1], F32, isOutput=False)
    d_c0 = nc.declare_dram_parameter("c0", [STATE, 1], F32, isOutput=False)
    d_selEO = nc.declare_dram_parameter("selEO", [2, 2], BF16, isOutput=False)
    d_S = nc.declare_dram_parameter("Sout", [1, T], F32, isOutput=True)
    d_sel = nc.declare_dram_parameter("selout", [1, T], F32, isOutput=True)
    if exchange == "cc":
        ccin = [nc.dram_tensor(f"ccin{i}", [128, 3], F32) for i in range(2)]
        ccout = [nc.dram_tensor(f"ccout{i}", [1024, 3], F32,
                                addr_space="Shared") for i in range(2)]

    # parity-split remote sems: step t uses rsems[t%2] so a wait can only be
    # satisfied by same-parity arrivals (peers can be at most 1 step ahead,
    # which is the other parity — no overshoot race).
    rsems = [nc.alloc_semaphore("rdma_rsem0"), nc.alloc_semaphore("rdma_rsem1")]
    lsems = [nc.alloc_semaphore("rdma_lsem0"), nc.alloc_semaphore("rdma_lsem1")]

    with tile.TileContext(nc) as tc, ExitStack() as ctxs:
        P = ctxs.enter_context(tc.tile_pool(name="static", bufs=1))
        W = ctxs.enter_context(tc.tile_pool(name="work", bufs=wbufs))
        PS = ctxs.enter_context(tc.tile_pool(name="psum", bufs=1,
                                             space="PSUM"))

        # ---------------- static SBUF tiles ----------------
        imB1 = P.tile([100, Lc], F32, tag="imB1")
        imB2 = P.tile([100, Lc], F32, tag="imB2")
        imTa = P.tile([128, NCH * 208], BF16, tag="imTa")
        w1Ta = P.tile([100, ATT], F32, tag="w1Ta")
        w1Tb = P.tile([100, ATT], F32, tag="w1Tb")
        vb = P.tile([ATT, 1], BF16, tag="vb")
        Wg_ctxA = P.tile([128, 400], F32, tag="WgcA")
        Wg_ctxB = P.tile([72, 400], F32, tag="WgcB")
        Wg_hb = P.tile([101, 400], F32, tag="Wghb")
        w2Th = P.tile([100, ATT], F32, tag="w2Th")
        w2Tc = P.tile([100, ATT], F32, tag="w2Tc")
        linTb = P.tile([101, VOCAB], F32, tag="linTb")
        linsel = P.tile([101, T], F32, tag="linsel")
        gEmb = P.tile([100, 4 * T], F32, tag="gEmb")
        h_aug = P.tile([101, 1], F32, tag="haug")
        c_sb = P.tile([STATE, 1], F32, tag="c")
        w1tb = P.tile([ATT, Lc], BF16, tag="w1tb")
        tanh_sb = P.tile([ATT, Lc], BF16, tag="tanhsb")
        Sbuf = P.tile([1, T], F32, tag="Sbuf")
        selbuf = P.tile([1, T], F32, tag="selbuf")
        selEO = P.tile([2, 2], BF16, tag="selEO")
        comm = [P.tile([128, 3], F32, tag=f"comm{i}", name=f"comm{i}")
                for i in range(2)]
        gather = [P.tile([128, 24], F32, tag=f"gather{i}", name=f"gather{i}")
                  for i in range(2)]

        # ---------------- init ----------------
        nc.sync.dma_start(imB1[:], d_imB[0:100, :])
        nc.sync.dma_start(imB2[:], d_imB[100:200, :])
        nc.sync.dma_start(imTa[:], d_imTa[:])
        nc.sync.dma_start(w1Ta[:], d_w1T[0:100, :])
        nc.sync.dma_start(w1Tb[:], d_w1T[100:200, :])
        nc.sync.dma_start(vb[:], d_vb[:])
        nc.sync.dma_start(Wg_ctxA[:], d_Wg[0:128, :])
        nc.sync.dma_start(Wg_ctxB[:], d_Wg[128:200, :])
        nc.sync.dma_start(Wg_hb[:], d_Wg[200:301, :])
        nc.sync.dma_start(w2Th[:], d_w2T[0:100, :])
        nc.sync.dma_start(w2Tc[:], d_w2T[100:200, :])
        nc.sync.dma_start(linTb[:], d_linTb[:])
        nc.sync.dma_start(linsel[:], d_linsel[:])
        nc.sync.dma_start(gEmb[:], d_gEmb[:])
        nc.sync.dma_start(h_aug[:], d_h0[:])
        nc.sync.dma_start(c_sb[:], d_c0[:])
        nc.sync.dma_start(selEO[:], d_selEO[:])
        nc.gpsimd.memset(comm[0][:], 0.0)
        nc.gpsimd.memset(comm[1][:], 0.0)
        nc.gpsimd.memset(Sbuf[:], 1.0)
        nc.gpsimd.memset(selbuf[:], 0.0)

        # w1t = w1 @ input_mat   -> [ATT, Lc] bf16
        for j in range(Lc // 512):
            w1p = PS.tile([ATT, 512], F32, tag="w1p")
            sl = slice(512 * j, 512 * (j + 1))
            nc.tensor.matmul(w1p[:], w1Ta[:], imB1[:, sl], start=True,
                             stop=False)
            nc.tensor.matmul(w1p[:], w1Tb[:], imB2[:, sl], start=False,
                             stop=True)
            nc.scalar.copy(w1tb[:, sl], w1p[:])

        CH = NCH // n_tanh_chunks  # l-chunks per tanh chunk
        sE, sO = selEO[0:2, 0:1], selEO[0:2, 1:2]

        # ---------------- decode steps ----------------
        step = 0
        for t in [tt for _ in range(repeats) for tt in range(T)]:
            p = step % 2
            # w2dt = w2 @ [h; c]  -> bias for tanh (read directly from PSUM)
            w2p_full = PS.tile([ATT, 512], F32, tag="w2p")
            w2p = w2p_full[:, 0:1]
            nc.tensor.matmul(w2p[:], w2Th[:], h_aug[0:100, 0:1], start=True,
                             stop=False)
            nc.tensor.matmul(w2p[:], w2Tc[:], c_sb[:], start=False, stop=True)
            bias_sb = W.tile([ATT, 1], F32, tag="bias")
            nc.scalar.copy(bias_sb[:], w2p[:])

            scores_full = PS.tile([128, 512], F32, tag="scores")
            scores_ps = scores_full[:, 0:NCH]
            att_sb = W.tile([128, NCH], BF16, tag="att")
            ctx_full = PS.tile([2, 512], F32, tag="ctx")
            ctx_ps = ctx_full[:, 0:416]

            for ch in range(n_tanh_chunks):
                lo, hi = ch * CH, (ch + 1) * CH
                tlo, thi = (lo, lo + 128) if "tanh1" in skip else (lo * 128,
                                                                   hi * 128)
                nc.scalar.activation(tanh_sb[:, tlo:thi], w1tb[:, tlo:thi],
                                     Tanh, bias=bias_sb[:, 0:1])
                srange = [lo] if "scores1" in skip else range(lo, hi)
                for c in srange:
                    nc.tensor.matmul(scores_ps[:, c:c + 1],
                                     tanh_sb[:, c * 128:(c + 1) * 128],
                                     vb[:], start=True, stop=True)
                nc.scalar.activation(att_sb[:, lo:hi], scores_ps[:, lo:hi],
                                     Exp)
                crange = [lo] if "ctx4" in skip else range(lo, hi, 2)
                for c in crange:
                    nc.tensor.matmul(ctx_ps[:],
                                     att_sb[:, c:c + 2],
                                     imTa[:, c * 208:(c + 2) * 208],
                                     start=(c == lo if "ctx4" in skip
                                            else c == 0),
                                     stop=(c >= (hi if "ctx4" in skip
                                                 else NCH) - 2))

            if "post" in skip:
                step += 1
                continue

            # partial [2, 416] -> column layout [128, 4] (reduce E/O + transpose)
            # cu columns: 0 = num[0:128], 1 = den (bcast), 2 = num[128:200]
            # (den in the middle so the initialized region is col 0:2 full
            # plus col 2 rows 0:72 — contiguous slices for the comm copy)
            num_sb = W.tile([2, 416], BF16, tag="num")
            nc.vector.tensor_copy(num_sb[:], ctx_ps[:])
            cu_full = PS.tile([128, 512], F32, tag="cu")
            cu = cu_full[:, 0:3]
            nc.tensor.matmul(cu[:, 0:1], num_sb[:, 1:129], sE,
                             start=True, stop=False)
            nc.tensor.matmul(cu[:, 0:1], num_sb[:, 209:337], sO,
                             start=False, stop=True)
            nc.tensor.matmul(cu[:, 1:2], num_sb[:, 0:1].to_broadcast((2, 128)),
                             sE, start=True, stop=False)
            nc.tensor.matmul(cu[:, 1:2],
                             num_sb[:, 208:209].to_broadcast((2, 128)),
                             sO, start=False, stop=True)
            nc.tensor.matmul(cu[0:72, 2:3], num_sb[:, 129:201], sE,
                             start=True, stop=False)
            nc.tensor.matmul(cu[0:72, 2:3], num_sb[:, 337:409], sO,
                             start=False, stop=True)

            # ---- exchange: broadcast own [128, 4] partial to all 8 cores ----
            if exchange == "rdma":
                with tc.tile_critical(name="commw"):
                    # comm[p] reusable once step-(t-2) sends (same parity,
                    # 8 broadcasts x16 local_sem) completed
                    cp = nc.vector.tensor_copy(comm[p][:, 0:2], cu[:, 0:2])
                    if step >= 2:
                        # same-parity sends of step t-2 all completed locally
                        cp._wait_ge(lsems[p], 128 * (step // 2))
                    nc.vector.tensor_copy(comm[p][0:72, 2:3], cu[0:72, 2:3])
                # descgen runs early on Pool (no-sync deps); the comm-tile
                # read is deferred to trigger_dma, which waits for the copy
                for d in range(8):
                    nc.gpsimd.remote_dma_broadcast(
                        gather[p][:, 3 * d:3 * d + 3], comm[p][:],
                        rsems[p], lsems[p],
                        rdests=[(0, d) if k == d else None for k in range(8)])
                nc.gpsimd.trigger_dma(count=None)
                g = gather[p]
                r12 = W.tile([128, 12], F32, tag="r12")
                with tc.tile_critical(name="redw"):
                    # gate the first reduce on all 8 peers' arrivals
                    a1 = nc.vector.tensor_tensor(r12[:], g[:, 0:12],
                                                 g[:, 12:24], op=ADD)
                    a1._wait_ge(rsems[p], 16 * (step // 2 + 1))
            elif exchange == "cc":
                commt = W.tile([128, 3], F32, tag="commt")
                nc.scalar.activation(commt[:, 0:2], cu[:, 0:2], Copy)
                nc.scalar.activation(commt[0:72, 2:3], cu[0:72, 2:3], Copy)
                nc.gpsimd.memset(commt[72:128, 2:3], 0.0)
                cin, cout = ccin[p], ccout[p]
                nc.sync.dma_start(cin[:], commt[:])
                nc.gpsimd.collective_compute(
                    "AllGather", mybir.AluOpType.bypass,
                    replica_groups=[list(range(NCORES))],
                    ins=[cin.ap().opt()], outs=[cout.ap().opt()])
                g = W.tile([128, 24], F32, tag="gatherw")
                for d in range(8):
                    nc.sync.dma_start(g[:, 3 * d:3 * d + 3],
                                      cout[128 * d:128 * (d + 1), :])
            else:  # exchange == "none" (ablation: pretend sum == 8x local)
                g = W.tile([128, 24], F32, tag="gatherw")
                nc.gpsimd.memset(g[:], 0.0)
                for d in range(8):
                    nc.scalar.activation(g[:, 3 * d:3 * d + 2], cu[:, 0:2],
                                         Copy)

            # reduce the 8 slots -> [128, 3]; cols: ctxA, ctxB, den
            if exchange != "rdma":
                r12 = W.tile([128, 12], F32, tag="r12")
                nc.vector.tensor_tensor(r12[:], g[:, 0:12], g[:, 12:24],
                                        op=ADD)
            r6 = W.tile([128, 6], F32, tag="r6")
            nc.vector.tensor_tensor(r6[:], r12[:, 0:6], r12[:, 6:12], op=ADD)
            cu_tot = W.tile([128, 3], F32, tag="cutot")
            nc.vector.tensor_tensor(cu_tot[:], r6[:, 0:3], r6[:, 3:6], op=ADD)
            rd = W.tile([128, 1], F32, tag="rd")
            nc.vector.reciprocal(rd[:], cu_tot[:, 1:2])
            ctx_sb = W.tile([128, 3], F32, tag="ctxs")
            nc.scalar.activation(ctx_sb[:], cu_tot[:], Copy,
                                 scale=rd[:, 0:1])

            # gates = Wg @ [ctx(200); h; 1]  -> [100, 4] (i,f,o pre-scaled .5)
            gates_full = PS.tile([100, 512], F32, tag="gates")
            gates_ps = gates_full[:, 0:4]
            for gi in range(4 if "gates" not in skip else 0):
                gs = slice(100 * gi, 100 * (gi + 1))
                nc.tensor.matmul(gates_ps[:, gi:gi + 1], Wg_ctxA[:, gs],
                                 ctx_sb[:, 0:1], start=True, stop=False)
                nc.tensor.matmul(gates_ps[:, gi:gi + 1], Wg_ctxB[:, gs],
                                 ctx_sb[0:72, 2:3], start=False, stop=False)
                nc.tensor.matmul(gates_ps[:, gi:gi + 1], Wg_hb[:, gs],
                                 h_aug[:], start=False, stop=True)

            if "lstm" in skip:
                step += 1
                continue
            # LSTM elementwise; c_sb/h hold 2x-scaled state
            gsum = W.tile([100, 4], F32, tag="gsum")
            nc.vector.tensor_tensor(gsum[:], gates_ps[:],
                                    gEmb[:, 4 * t:4 * t + 4], op=ADD)
            t_all = W.tile([100, 4], F32, tag="tall")
            nc.scalar.activation(t_all[:], gsum[:], Tanh)
            tA = W.tile([100, 1], F32, tag="tA")
            nc.vector.scalar_tensor_tensor(tA[:], t_all[:, 1:2], 1.0, c_sb[:],
                                           ADD, MULT)
            tB = W.tile([100, 1], F32, tag="tB")
            nc.vector.scalar_tensor_tensor(tB[:], t_all[:, 0:1], 1.0,
                                           t_all[:, 3:4], ADD, MULT)
            nc.vector.scalar_tensor_tensor(c_sb[:], tA[:], 0.5, tB[:],
                                           MULT, ADD)
            tanh_c = W.tile([100, 1], F32, tag="tanhc")
            nc.scalar.activation(tanh_c[:], c_sb[:], Tanh, scale=0.5)
            nc.vector.scalar_tensor_tensor(h_aug[0:100, 0:1], t_all[:, 2:3],
                                           1.0, tanh_c[:], ADD, MULT)

            # logits + per-step loss pieces (off the h/c critical path)
            if "logits" not in skip:
                lg_full = PS.tile([1, 512], F32, tag="lg")
                lg_ps = lg_full[:, 0:129]
                nc.tensor.matmul(lg_ps[0:1, 0:128], h_aug[:, 0:1], linTb[:],
                                 start=True, stop=True)
                nc.tensor.matmul(lg_ps[0:1, 128:129], h_aug[:, 0:1],
                                 linsel[:, t:t + 1], start=True, stop=True)
                exps = W.tile([1, VOCAB], F32, tag="exps")
                nc.scalar.activation(exps[:], lg_ps[0:1, 0:128], Exp,
                                     accum_out=Sbuf[0:1, t:t + 1])
                nc.vector.tensor_copy(selbuf[0:1, t:t + 1],
                                      lg_ps[0:1, 128:129])
            step += 1

        nc.sync.dma_start(d_S[:], Sbuf[:])
        nc.sync.dma_start(d_sel[:], selbuf[:])

    return nc


# =================== host preprocessing ===================

def _lstm_step_np(x, h, c, W_ih, W_hh, b_ih, b_hh):
    gates = W_ih @ x + b_ih + W_hh @ h + b_hh
    i, f, g, o = np.split(gates, 4)
    sig = lambda v: 1.0 / (1.0 + np.exp(-v))
    c = sig(f) * c + sig(i) * np.tanh(g)
    h = sig(o) * np.tanh(c)
    return h, c


def prep_inputs(inputs, Lc, T):
    """Produce the 8 per-core in_maps from the full problem inputs."""
    im = np.asarray(inputs["input_mat"], np.float32)        # [200, L]
    output_ids = np.asarray(inputs["output_ids"]).astype(np.int64)
    W_ih = np.asarray(inputs["W_ih"], np.float32)
    W_hh = np.asarray(inputs["W_hh"], np.float32)
    b_ih = np.asarray(inputs["b_ih"], np.float32)
    b_hh = np.asarray(inputs["b_hh"], np.float32)
    w1 = np.asarray(inputs["w1"], np.float32)
    w2 = np.asarray(inputs["w2"], np.float32)
    v_w = np.asarray(inputs["v_w"], np.float32)
    lin_w = np.asarray(inputs["lin_w"], np.float32)
    lin_b = np.asarray(inputs["lin_b"], np.float32)
    emb = np.asarray(inputs["emb"], np.float32)
    eos = int(np.asarray(inputs["eos_id"]))

    L = im.shape[1]
    assert Lc * NCORES == L and len(output_ids) == T
    NCH = Lc // 128

    # priming LSTM step on host (exact fp32 math, tiny); store 2x-scaled
    x0 = np.concatenate([np.zeros(200, np.float32), emb[eos]])
    h0, c0 = _lstm_step_np(x0, np.zeros(100, np.float32),
                           np.zeros(100, np.float32), W_ih, W_hh, b_ih, b_hh)
    h0aug = np.concatenate([2.0 * h0, [1.0]]).astype(np.float32)
    h0aug = h0aug.reshape(101, 1)
    c0s = (2.0 * c0).reshape(100, 1).astype(np.float32)

    # gate weights: reorder [i,f,g,o] -> [i,f,o,g]; scale i,f,o rows by 0.5
    # (tanh-form sigmoid); h columns additionally by 0.5 (h,c stored 2x).
    Wcomb = np.concatenate([W_ih, W_hh], axis=1)            # [400, 400]
    bias = (b_ih + b_hh).astype(np.float32)                 # [400]
    order = np.concatenate([np.arange(100), np.arange(100, 200),
                            np.arange(300, 400), np.arange(200, 300)])
    Wr = Wcomb[order]                                       # rows i,f,o,g
    br = bias[order].copy()
    rscale = np.ones((400, 1), np.float32); rscale[0:300] = 0.5
    Wr = Wr * rscale; br = br * rscale[:, 0]
    Wr[:, 300:400] *= 0.5                                   # h stored as 2h
    # emb-path gate preactivations per step: [100, 4] blocks (i,f,o,g cols)
    emb_seq = np.empty((T, EMB), np.float32)
    emb_seq[0] = emb[eos]
    emb_seq[1:] = emb[output_ids[:T - 1]]
    gEmbAll = emb_seq @ Wr[:, 200:300].T + br               # [T, 400]
    gEmb = gEmbAll.reshape(T, 4, 100).transpose(2, 0, 1)    # [100, T, 4]
    gEmb = np.ascontiguousarray(gEmb.reshape(100, 4 * T), np.float32)
    # device-side gates input x = [ctx(200); h2(100); 1]
    Wg = np.zeros((301, 400), np.float32)
    Wg[0:200] = Wr[:, 0:200].T       # ctx
    Wg[200:300] = Wr[:, 300:400].T   # h (2x-scaled)
    Wg[300] = 0.0                    # bias handled in gEmb
    # logits weights; h rows scaled 0.5 (h stored 2x)
    linTb = np.concatenate([0.5 * lin_w.T, lin_b.reshape(1, -1)], axis=0)
    linsel = np.concatenate([0.5 * lin_w[output_ids[:T]].T,
                             lin_b[output_ids[:T]].reshape(1, -1)], axis=0)

    w1T = w1.T.copy()                                  # [200, 100]
    w2T = (0.5 * w2.T).copy()                          # [200, 100]; h,c are 2x
    vbf = v_w.reshape(ATT, 1).astype(ml_dtypes.bfloat16)
    selEO = np.zeros((2, 2), np.float32)
    selEO[0, 0] = 1.0
    selEO[1, 1] = 1.0

    in_maps = []
    for cidx in range(NCORES):
        sl = slice(cidx * Lc, (cidx + 1) * Lc)
        imc = im[:, sl]                                    # [200, Lc]
        # imTa[q, c*208 + s] = im[s, c*128+q]; col 0 = 1.0 (den)
        blocks = imc.T.reshape(NCH, 128, 200)              # [c, q, s]
        imTa = np.concatenate(
            [np.ones((NCH, 128, 1), np.float32), blocks,
             np.zeros((NCH, 128, 7), np.float32)], axis=2)  # [c,q,208]
        imTa = imTa.transpose(1, 0, 2).reshape(128, NCH * 208)
        in_maps.append({
            "selEO": selEO.astype(ml_dtypes.bfloat16),
            "imB": np.ascontiguousarray(imc),
            "imTa": np.ascontiguousarray(imTa).astype(ml_dtypes.bfloat16),
            "w1T": w1T, "vb": vbf, "Wg": Wg, "w2T": w2T,
            "linTb": linTb.astype(np.float32),
            "linsel": linsel.astype(np.float32),
            "gEmb": gEmb,
            "h0aug": h0aug, "c0": c0s,
        })
    return in_maps


def finish_loss(Sout, selout):
    """loss = sum_t ( log(sum_j exp(logit_j)) - logit_sel )."""
    S = np.asarray(Sout, np.float64).ravel()
    sel = np.asarray(selout, np.float64).ravel()
    return np.float32(np.sum(np.log(S) - sel))


# =================== self-contained runner ===================
LC = 8192
T_STEPS = 258
_CACHE = {}


def _get_compiled():
    if "nc" not in _CACHE:
        nc = build_kernel(LC, T_STEPS)
        nc.compile()
        _CACHE["nc"] = nc
    return _CACHE["nc"]


def kernel(**inputs):
    """Full-input AttnLSTM decoder loss on 8 trn2 cores."""
    from concourse import bass_utils
    nc = _get_compiled()
    in_maps = prep_inputs(inputs, LC, T_STEPS)
    res = bass_utils.run_bass_kernel_spmd(nc, in_maps,
                                          core_ids=list(range(NCORES)))
    out = res.results[0]
    return np.asarray(finish_loss(out["Sout"], out["selout"]))
